# revision 1
# baseline (speedup 1.0000x reference)
"""Trainium2 Bass kernel for nn_Classifier (subdom hinge loss + 4-layer MLP).

Data-parallel over 8 NeuronCores: batch rows of x and demo rows of
sample_loss/demo_metric are sharded; MLP weights are replicated.

Device layout: activations kept transposed ([feature, batch]) so the weight
matrices are the natural stationary (lhsT) operand of the tensor engine and
no on-device transposes are needed.  Matmuls run in bf16 with fp32 PSUM
accumulation; the hinge loss runs in fp32 on the vector/scalar engines with
a fused relu+row-sum (accum_out).
"""

import numpy as np
import ml_dtypes
from contextlib import ExitStack

# Problem sizes (hardcoded per the task spec).
B, F, H = 16384, 2048, 4096
D, K = 4096, 2048
NCORES = 8
M = B // NCORES          # 2048 batch rows per core
DS = D // NCORES         # 512 demo rows per core
P = 128
FD = 512                 # matmul moving free dim == m-chunk size
NMC = M // FD            # 4 m-chunks
KT1 = F // P             # 16 k-tiles, layer 1
NT = H // P              # 32 n-tiles (and k-tiles for layers 2/3)
KLT = K // P             # 16 k-tiles of the demo feature dim

BF16 = ml_dtypes.bfloat16

_CACHE = {}


def _build():
    import concourse.tile as tile
    from concourse import bacc, mybir

    f32 = mybir.dt.float32
    bf16 = mybir.dt.bfloat16
    Act = mybir.ActivationFunctionType

    nc = bacc.Bacc("TRN2", target_bir_lowering=False, debug=False,
                   num_devices=NCORES)

    xT_d = nc.dram_tensor("xT", [F, M], bf16, kind="ExternalInput")
    w1_d = nc.dram_tensor("w1", [F, H], bf16, kind="ExternalInput")
    w2_d = nc.dram_tensor("w2", [H, H], bf16, kind="ExternalInput")
    w3_d = nc.dram_tensor("w3", [H, H], bf16, kind="ExternalInput")
    w4_d = nc.dram_tensor("w4", [H, 2], bf16, kind="ExternalInput")
    # packed consts: cols 0:32 b1, 32:64 b2, 64:96 b3, 96:112 alpha, 112 b4
    cst_d = nc.dram_tensor("cst", [P, 113], f32, kind="ExternalInput")
    slT_d = nc.dram_tensor("slT", [K, DS], f32, kind="ExternalInput")
    dmT_d = nc.dram_tensor("dmT", [K, DS], f32, kind="ExternalInput")
    probsT_d = nc.dram_tensor("probsT", [2, M], f32, kind="ExternalOutput")
    lpart_d = nc.dram_tensor("lpart", [P, 1], f32, kind="ExternalOutput")

    with tile.TileContext(nc) as tc, ExitStack() as ctx:
        const = ctx.enter_context(tc.tile_pool(name="const", bufs=1))
        cst = const.tile([P, 113], f32, name="cst_sb")
        nc.sync.dma_start(cst[:], cst_d.ap()[:])
        w4t = const.tile([P, NT, 2], bf16, name="w4_sb")
        nc.sync.dma_start(w4t[:], w4_d.ap().rearrange("(kt p) n -> p kt n", p=P))
        acc = const.tile([P, KLT], f32, name="acc_sb")

        # ---- subdom hinge loss: sum over relu(alpha*(sl-dm)+1), fp32 ----
        lpool = ctx.enter_context(tc.tile_pool(name="loss", bufs=2))
        for j in range(KLT):
            slt = lpool.tile([P, DS], f32, name=f"sl_{j}", tag="sl")
            nc.sync.dma_start(slt[:], slT_d.ap()[j * P:(j + 1) * P, :])
            dmt = lpool.tile([P, DS], f32, name=f"dm_{j}", tag="dm")
            nc.sync.dma_start(dmt[:], dmT_d.ap()[j * P:(j + 1) * P, :])
            nc.vector.tensor_sub(slt[:], slt[:], dmt[:])
            scr = lpool.tile([P, DS], f32, name=f"scr_{j}", tag="scr")
            nc.scalar.activation(scr[:], slt[:], Act.Relu, bias=1.0,
                                 scale=cst[:, 96 + j:97 + j],
                                 accum_out=acc[:, j:j + 1])
        lsum = const.tile([P, 1], f32, name="lsum_sb")
        nc.vector.reduce_sum(lsum[:], acc[:], axis=mybir.AxisListType.X)
        nc.sync.dma_start(lpart_d.ap()[:], lsum[:])

        # ---- MLP: 4 m-chunks of 512 batch cols ----
        xpool = ctx.enter_context(tc.tile_pool(name="xp", bufs=2))
        hpool = ctx.enter_context(tc.tile_pool(name="hp", bufs=2))
        wpool = ctx.enter_context(tc.tile_pool(name="wp", bufs=3))
        ppool = ctx.enter_context(tc.tile_pool(name="pp", bufs=4, space="PSUM"))
        p4pool = ctx.enter_context(tc.tile_pool(name="p4", bufs=2, space="PSUM"))
        opool = ctx.enter_context(tc.tile_pool(name="op", bufs=2))

        layers = [(w1_d, KT1, 0), (w2_d, NT, 32), (w3_d, NT, 64)]
        for mc in range(NMC):
            msl = slice(mc * FD, (mc + 1) * FD)
            xt = xpool.tile([P, KT1, FD], bf16, name=f"x_{mc}", tag="x")
            nc.sync.dma_start(
                xt[:], xT_d.ap().rearrange("(kt p) m -> p kt m", p=P)[:, :, msl])
            hin = xt
            for li, (w_d, KT, boff) in enumerate(layers):
                hout = hpool.tile([P, NT, FD], bf16, name=f"h_{mc}_{li}", tag="h")
                for n in range(NT):
                    wt = wpool.tile([P, KT, P], bf16, name=f"w_{mc}_{li}_{n}",
                                    tag="w")
                    nc.sync.dma_start(
                        wt[:],
                        w_d.ap().rearrange("(kt p) n -> p kt n", p=P)
                        [:, :, n * P:(n + 1) * P])
                    ps = ppool.tile([P, FD], f32, name=f"ps_{mc}_{li}_{n}",
                                    tag="ps")
                    for k in range(KT):
                        nc.tensor.matmul(ps[:], wt[:, k, :], hin[:, k, :],
                                         start=(k == 0), stop=(k == KT - 1))
                    nc.scalar.activation(hout[:, n, :], ps[:], Act.Relu,
                                         bias=cst[:, boff + n:boff + n + 1])
                hin = hout
            ps4 = p4pool.tile([2, FD], f32, name=f"ps4_{mc}", tag="ps4")
            for k in range(NT):
                nc.tensor.matmul(ps4[:], w4t[:, k, :], hin[:, k, :],
                                 start=(k == 0), stop=(k == NT - 1))
            ot = opool.tile([2, FD], f32, name=f"o_{mc}", tag="o")
            nc.scalar.activation(ot[:], ps4[:], Act.Sigmoid,
                                 bias=cst[0:2, 112:113])
            nc.sync.dma_start(probsT_d.ap()[:, msl], ot[:])

    nc.compile()
    return nc


def _prep_in_maps(inputs):
    x = np.asarray(inputs["x"], dtype=np.float32)
    sl = np.asarray(inputs["sample_loss"], dtype=np.float32)
    dm = np.asarray(inputs["demo_metric"], dtype=np.float32)
    alpha = np.asarray(inputs["alpha"], dtype=np.float32)
    b4 = np.asarray(inputs["b4"], dtype=np.float32)

    w1 = np.asarray(inputs["W1"], dtype=np.float32).astype(BF16)
    w2 = np.asarray(inputs["W2"], dtype=np.float32).astype(BF16)
    w3 = np.asarray(inputs["W3"], dtype=np.float32).astype(BF16)
    w4 = np.asarray(inputs["W4"], dtype=np.float32).astype(BF16)

    cst = np.zeros((P, 113), np.float32)
    for col, bname in ((0, "b1"), (32, "b2"), (64, "b3")):
        bv = np.asarray(inputs[bname], dtype=np.float32)
        cst[:, col:col + bv.size // P] = bv.reshape(-1, P).T
    cst[:, 96:112] = alpha.reshape(-1, P).T
    cst[0:2, 112] = b4

    in_maps = []
    for c in range(NCORES):
        xT = np.ascontiguousarray(x[c * M:(c + 1) * M].T).astype(BF16)
        slT = np.ascontiguousarray(sl[c * DS:(c + 1) * DS].T)
        dmT = np.ascontiguousarray(dm[c * DS:(c + 1) * DS].T)
        in_maps.append({
            "xT": xT, "w1": w1, "w2": w2, "w3": w3, "w4": w4,
            "cst": cst, "slT": slT, "dmT": dmT,
        })
    return in_maps


def kernel(**inputs):
    from concourse.bass_utils import run_bass_kernel_spmd

    if "nc" not in _CACHE:
        _CACHE["nc"] = _build()
    nc = _CACHE["nc"]

    in_maps = _prep_in_maps(inputs)
    res = run_bass_kernel_spmd(nc, in_maps, core_ids=list(range(NCORES)))

    probs = np.concatenate(
        [res.results[c]["probsT"].T for c in range(NCORES)], axis=0)
    part = sum(float(res.results[c]["lpart"].sum(dtype=np.float64))
               for c in range(NCORES))
    c0 = float(np.asarray(inputs["subdom_constant"]).reshape(-1)[0])
    loss = np.float32(part - c0 * D * K)
    return loss, probs.astype(np.float32)


# revision 9
# speedup vs baseline: 5428.2263x; 5428.2263x over previous
"""Trainium2 Bass kernel for nn_Classifier (subdom hinge loss + 4-layer MLP).

Data-parallel over 8 NeuronCores: batch rows of x and demo rows of
sample_loss/demo_metric are sharded; MLP weights are replicated.

Device layout: activations kept transposed ([feature, batch]) so the weight
matrices are the natural stationary (lhsT) operand of the tensor engine and
no on-device transposes are needed.  Matmuls run in bf16 with fp32 PSUM
accumulation; the hinge loss runs in fp32 on the vector/scalar engines with
a fused relu+row-sum (accum_out).
"""

import numpy as np
import ml_dtypes
from contextlib import ExitStack

# Problem sizes (hardcoded per the task spec).
B, F, H = 16384, 2048, 4096
D, K = 4096, 2048
NCORES = 8
M = B // NCORES          # 2048 batch rows per core
DS = D // NCORES         # 512 demo rows per core
P = 128
FD = 512                 # matmul moving free dim == m-chunk size
NMC = M // FD            # 4 m-chunks
KT1 = F // P             # 16 k-tiles, layer 1
NT = H // P              # 32 n-tiles (and k-tiles for layers 2/3)
KLT = K // P             # 16 k-tiles of the demo feature dim

BF16 = ml_dtypes.bfloat16

_CACHE = {}


def _build(MC=1024, ms_inner=True, wbufs=3, xbufs=1, pbufs=4):
    import concourse.tile as tile
    from concourse import bacc, mybir

    NMCL = M // MC           # number of m-chunks
    MS = MC // FD            # 512-wide m-subchunks per chunk

    f32 = mybir.dt.float32
    bf16 = mybir.dt.bfloat16
    Act = mybir.ActivationFunctionType

    nc = bacc.Bacc("TRN2", target_bir_lowering=False, debug=False,
                   num_devices=NCORES)

    xT_d = nc.dram_tensor("xT", [F, M], bf16, kind="ExternalInput")
    w1_d = nc.dram_tensor("w1", [F, H], bf16, kind="ExternalInput")
    w2_d = nc.dram_tensor("w2", [H, H], bf16, kind="ExternalInput")
    w3_d = nc.dram_tensor("w3", [H, H], bf16, kind="ExternalInput")
    w4_d = nc.dram_tensor("w4", [H, 2], bf16, kind="ExternalInput")
    # packed consts: cols 0:32 b1, 32:64 b2, 64:96 b3, 96:112 alpha, 112 b4
    cst_d = nc.dram_tensor("cst", [P, 113], f32, kind="ExternalInput")
    slT_d = nc.dram_tensor("slT", [K, DS], f32, kind="ExternalInput")
    dmT_d = nc.dram_tensor("dmT", [K, DS], f32, kind="ExternalInput")
    probsT_d = nc.dram_tensor("probsT", [2, M], f32, kind="ExternalOutput")
    lpart_d = nc.dram_tensor("lpart", [P, 1], f32, kind="ExternalOutput")

    with tile.TileContext(nc) as tc, ExitStack() as ctx:
        const = ctx.enter_context(tc.tile_pool(name="const", bufs=1))
        cst = const.tile([P, 113], f32, name="cst_sb")
        nc.sync.dma_start(cst[:], cst_d.ap()[:])
        w4t = const.tile([P, NT, 2], bf16, name="w4_sb")
        nc.sync.dma_start(w4t[:], w4_d.ap().rearrange("(kt p) n -> p kt n", p=P))
        acc = const.tile([P, KLT], f32, name="acc_sb")

        # ---- subdom hinge loss: sum over relu(alpha*(sl-dm)+1), fp32 ----
        lpool = ctx.enter_context(tc.tile_pool(name="loss", bufs=2))
        for j in range(KLT):
            slt = lpool.tile([P, DS], f32, name=f"sl_{j}", tag="sl")
            nc.sync.dma_start(slt[:], slT_d.ap()[j * P:(j + 1) * P, :])
            dmt = lpool.tile([P, DS], f32, name=f"dm_{j}", tag="dm")
            nc.sync.dma_start(dmt[:], dmT_d.ap()[j * P:(j + 1) * P, :])
            nc.vector.tensor_sub(slt[:], slt[:], dmt[:])
            nc.scalar.activation(slt[:], slt[:], Act.Relu, bias=1.0,
                                 scale=cst[:, 96 + j:97 + j],
                                 accum_out=acc[:, j:j + 1])
        lsum = const.tile([P, 1], f32, name="lsum_sb")
        nc.vector.reduce_sum(lsum[:], acc[:], axis=mybir.AxisListType.X)
        nc.sync.dma_start(lpart_d.ap()[:], lsum[:])

        # ---- MLP: 4 m-chunks of 512 batch cols ----
        xpool = ctx.enter_context(tc.tile_pool(name="xp", bufs=xbufs))
        hpool = ctx.enter_context(tc.tile_pool(name="hp", bufs=2))
        wpool = ctx.enter_context(tc.tile_pool(name="wp", bufs=wbufs))
        ppool = ctx.enter_context(tc.tile_pool(name="pp", bufs=pbufs,
                                               space="PSUM"))
        p4pool = ctx.enter_context(tc.tile_pool(name="p4", bufs=2, space="PSUM"))
        opool = ctx.enter_context(tc.tile_pool(name="op", bufs=2))

        layers = [(w1_d, KT1, 0), (w2_d, NT, 32), (w3_d, NT, 64)]
        for mc in range(NMCL):
            xt = xpool.tile([P, KT1, MC], bf16, name=f"x_{mc}", tag="x")
            nc.sync.dma_start(
                xt[:], xT_d.ap().rearrange("(kt p) m -> p kt m", p=P)
                [:, :, mc * MC:(mc + 1) * MC])
            hin = xt
            for li, (w_d, KT, boff) in enumerate(layers):
                hout = hpool.tile([P, NT, MC], bf16, name=f"h_{mc}_{li}",
                                  tag="h")
                for n in range(NT):
                    wt = wpool.tile([P, KT, P], bf16, name=f"w_{mc}_{li}_{n}",
                                    tag="w")
                    nc.sync.dma_start(
                        wt[:],
                        w_d.ap().rearrange("(kt p) n -> p kt n", p=P)
                        [:, :, n * P:(n + 1) * P])
                    pss = [ppool.tile([P, FD], f32,
                                      name=f"ps_{mc}_{li}_{n}_{ms}", tag="ps")
                           for ms in range(MS)]
                    if ms_inner:
                        for k in range(KT):
                            for ms in range(MS):
                                nc.tensor.matmul(
                                    pss[ms][:], wt[:, k, :],
                                    hin[:, k, ms * FD:(ms + 1) * FD],
                                    start=(k == 0), stop=(k == KT - 1))
                    else:
                        for ms in range(MS):
                            for k in range(KT):
                                nc.tensor.matmul(
                                    pss[ms][:], wt[:, k, :],
                                    hin[:, k, ms * FD:(ms + 1) * FD],
                                    start=(k == 0), stop=(k == KT - 1))
                    for ms in range(MS):
                        nc.scalar.activation(
                            hout[:, n, ms * FD:(ms + 1) * FD], pss[ms][:],
                            Act.Relu, bias=cst[:, boff + n:boff + n + 1])
                hin = hout
            for ms in range(MS):
                ps4 = p4pool.tile([2, FD], f32, name=f"ps4_{mc}_{ms}",
                                  tag="ps4")
                for k in range(NT):
                    nc.tensor.matmul(ps4[:], w4t[:, k, :],
                                     hin[:, k, ms * FD:(ms + 1) * FD],
                                     start=(k == 0), stop=(k == NT - 1))
                ot = opool.tile([2, FD], f32, name=f"o_{mc}_{ms}", tag="o")
                nc.scalar.activation(ot[:], ps4[:], Act.Sigmoid,
                                     bias=cst[0:2, 112:113])
                nc.sync.dma_start(
                    probsT_d.ap()[:, mc * MC + ms * FD:mc * MC + (ms + 1) * FD],
                    ot[:])

    nc.compile()
    return nc


def _prep_in_maps(inputs):
    x = np.asarray(inputs["x"], dtype=np.float32)
    sl = np.asarray(inputs["sample_loss"], dtype=np.float32)
    dm = np.asarray(inputs["demo_metric"], dtype=np.float32)
    alpha = np.asarray(inputs["alpha"], dtype=np.float32)
    b4 = np.asarray(inputs["b4"], dtype=np.float32)

    w1 = np.asarray(inputs["W1"], dtype=np.float32).astype(BF16)
    w2 = np.asarray(inputs["W2"], dtype=np.float32).astype(BF16)
    w3 = np.asarray(inputs["W3"], dtype=np.float32).astype(BF16)
    w4 = np.asarray(inputs["W4"], dtype=np.float32).astype(BF16)

    cst = np.zeros((P, 113), np.float32)
    for col, bname in ((0, "b1"), (32, "b2"), (64, "b3")):
        bv = np.asarray(inputs[bname], dtype=np.float32)
        cst[:, col:col + bv.size // P] = bv.reshape(-1, P).T
    cst[:, 96:112] = alpha.reshape(-1, P).T
    cst[0:2, 112] = b4

    in_maps = []
    for c in range(NCORES):
        xT = np.ascontiguousarray(x[c * M:(c + 1) * M].T).astype(BF16)
        slT = np.ascontiguousarray(sl[c * DS:(c + 1) * DS].T)
        dmT = np.ascontiguousarray(dm[c * DS:(c + 1) * DS].T)
        in_maps.append({
            "xT": xT, "w1": w1, "w2": w2, "w3": w3, "w4": w4,
            "cst": cst, "slT": slT, "dmT": dmT,
        })
    return in_maps


def kernel(**inputs):
    from concourse.bass_utils import run_bass_kernel_spmd

    if "nc" not in _CACHE:
        _CACHE["nc"] = _build()
    nc = _CACHE["nc"]

    in_maps = _prep_in_maps(inputs)
    res = run_bass_kernel_spmd(nc, in_maps, core_ids=list(range(NCORES)))

    probs = np.concatenate(
        [res.results[c]["probsT"].T for c in range(NCORES)], axis=0)
    part = sum(float(res.results[c]["lpart"].sum(dtype=np.float64))
               for c in range(NCORES))
    c0 = float(np.asarray(inputs["subdom_constant"]).reshape(-1)[0])
    loss = np.float32(part - c0 * D * K)
    return loss, probs.astype(np.float32)


# revision 13
# speedup vs baseline: 5467.9428x; 1.0073x over previous
"""Trainium2 Bass kernel for nn_Classifier (subdom hinge loss + 4-layer MLP).

Data-parallel over 8 NeuronCores: batch rows of x and demo rows of
sample_loss/demo_metric are sharded; MLP weights are replicated.

Device layout: activations kept transposed ([feature, batch]) so the weight
matrices are the natural stationary (lhsT) operand of the tensor engine and
no on-device transposes are needed.  Matmuls run in bf16 with fp32 PSUM
accumulation; the hinge loss runs in fp32 on the vector/scalar engines with
a fused relu+row-sum (accum_out).
"""

import numpy as np
import ml_dtypes
from contextlib import ExitStack

# Problem sizes (hardcoded per the task spec).
B, F, H = 16384, 2048, 4096
D, K = 4096, 2048
NCORES = 8
M = B // NCORES          # 2048 batch rows per core
DS = D // NCORES         # 512 demo rows per core
P = 128
FD = 512                 # matmul moving free dim == m-chunk size
NMC = M // FD            # 4 m-chunks
KT1 = F // P             # 16 k-tiles, layer 1
NT = H // P              # 32 n-tiles (and k-tiles for layers 2/3)
KLT = K // P             # 16 k-tiles of the demo feature dim

BF16 = ml_dtypes.bfloat16

_CACHE = {}


def _build(MC=1024, ms_inner=True, wbufs=3, xbufs=1, pbufs=4,
           split_x=True, loss_mid=True):
    import concourse.tile as tile
    from concourse import bacc, mybir

    NMCL = M // MC           # number of m-chunks
    MS = MC // FD            # 512-wide m-subchunks per chunk

    f32 = mybir.dt.float32
    bf16 = mybir.dt.bfloat16
    Act = mybir.ActivationFunctionType

    nc = bacc.Bacc("TRN2", target_bir_lowering=False, debug=False,
                   num_devices=NCORES)

    xT_d = nc.dram_tensor("xT", [F, M], bf16, kind="ExternalInput")
    w1_d = nc.dram_tensor("w1", [F, H], bf16, kind="ExternalInput")
    w2_d = nc.dram_tensor("w2", [H, H], bf16, kind="ExternalInput")
    w3_d = nc.dram_tensor("w3", [H, H], bf16, kind="ExternalInput")
    w4_d = nc.dram_tensor("w4", [H, 2], bf16, kind="ExternalInput")
    # packed consts: cols 0:32 b1, 32:64 b2, 64:96 b3, 96:112 alpha, 112 b4
    cst_d = nc.dram_tensor("cst", [P, 113], f32, kind="ExternalInput")
    slT_d = nc.dram_tensor("slT", [K, DS], f32, kind="ExternalInput")
    dmT_d = nc.dram_tensor("dmT", [K, DS], f32, kind="ExternalInput")
    probsT_d = nc.dram_tensor("probsT", [2, M], f32, kind="ExternalOutput")
    lpart_d = nc.dram_tensor("lpart", [P, 1], f32, kind="ExternalOutput")

    with tile.TileContext(nc) as tc, ExitStack() as ctx:
        const = ctx.enter_context(tc.tile_pool(name="const", bufs=1))
        cst = const.tile([P, 113], f32, name="cst_sb")
        nc.sync.dma_start(cst[:], cst_d.ap()[:])
        w4t = const.tile([P, NT, 2], bf16, name="w4_sb")
        nc.sync.dma_start(w4t[:], w4_d.ap().rearrange("(kt p) n -> p kt n", p=P))
        acc = const.tile([P, KLT], f32, name="acc_sb")

        # ---- subdom hinge loss: sum over relu(alpha*(sl-dm)+1), fp32 ----
        lpool = ctx.enter_context(tc.tile_pool(name="loss", bufs=2))

        def emit_loss():
            for j in range(KLT):
                slt = lpool.tile([P, DS], f32, name=f"sl_{j}", tag="sl")
                nc.sync.dma_start(slt[:], slT_d.ap()[j * P:(j + 1) * P, :])
                dmt = lpool.tile([P, DS], f32, name=f"dm_{j}", tag="dm")
                nc.sync.dma_start(dmt[:], dmT_d.ap()[j * P:(j + 1) * P, :])
                nc.vector.tensor_sub(slt[:], slt[:], dmt[:])
                nc.scalar.activation(slt[:], slt[:], Act.Relu, bias=1.0,
                                     scale=cst[:, 96 + j:97 + j],
                                     accum_out=acc[:, j:j + 1])
            lsum = const.tile([P, 1], f32, name="lsum_sb")
            nc.vector.reduce_sum(lsum[:], acc[:], axis=mybir.AxisListType.X)
            nc.sync.dma_start(lpart_d.ap()[:], lsum[:])

        if not loss_mid:
            emit_loss()

        # ---- MLP: 4 m-chunks of 512 batch cols ----
        xpool = ctx.enter_context(tc.tile_pool(name="xp", bufs=xbufs))
        hpool = ctx.enter_context(tc.tile_pool(name="hp", bufs=2))
        wpool = ctx.enter_context(tc.tile_pool(name="wp", bufs=wbufs))
        ppool = ctx.enter_context(tc.tile_pool(name="pp", bufs=pbufs,
                                               space="PSUM"))
        p4pool = ctx.enter_context(tc.tile_pool(name="p4", bufs=2, space="PSUM"))
        opool = ctx.enter_context(tc.tile_pool(name="op", bufs=2))

        layers = [(w1_d, KT1, 0), (w2_d, NT, 32), (w3_d, NT, 64)]
        for mc in range(NMCL):
            if loss_mid and mc == 1:
                emit_loss()
            xt = xpool.tile([P, KT1, MC], bf16, name=f"x_{mc}", tag="x")
            xsrc = (xT_d.ap().rearrange("(kt p) m -> p kt m", p=P)
                    [:, :, mc * MC:(mc + 1) * MC])
            if split_x:
                for kt in range(KT1):
                    nc.sync.dma_start(xt[:, kt, :], xsrc[:, kt, :])
            else:
                nc.sync.dma_start(xt[:], xsrc)
            hin = xt
            for li, (w_d, KT, boff) in enumerate(layers):
                hout = hpool.tile([P, NT, MC], bf16, name=f"h_{mc}_{li}",
                                  tag="h")
                for n in range(NT):
                    wt = wpool.tile([P, KT, P], bf16, name=f"w_{mc}_{li}_{n}",
                                    tag="w")
                    nc.sync.dma_start(
                        wt[:],
                        w_d.ap().rearrange("(kt p) n -> p kt n", p=P)
                        [:, :, n * P:(n + 1) * P])
                    pss = [ppool.tile([P, FD], f32,
                                      name=f"ps_{mc}_{li}_{n}_{ms}", tag="ps")
                           for ms in range(MS)]
                    if ms_inner:
                        for k in range(KT):
                            for ms in range(MS):
                                nc.tensor.matmul(
                                    pss[ms][:], wt[:, k, :],
                                    hin[:, k, ms * FD:(ms + 1) * FD],
                                    start=(k == 0), stop=(k == KT - 1))
                    else:
                        for ms in range(MS):
                            for k in range(KT):
                                nc.tensor.matmul(
                                    pss[ms][:], wt[:, k, :],
                                    hin[:, k, ms * FD:(ms + 1) * FD],
                                    start=(k == 0), stop=(k == KT - 1))
                    for ms in range(MS):
                        nc.scalar.activation(
                            hout[:, n, ms * FD:(ms + 1) * FD], pss[ms][:],
                            Act.Relu, bias=cst[:, boff + n:boff + n + 1])
                hin = hout
            for ms in range(MS):
                ps4 = p4pool.tile([2, FD], f32, name=f"ps4_{mc}_{ms}",
                                  tag="ps4")
                for k in range(NT):
                    nc.tensor.matmul(ps4[:], w4t[:, k, :],
                                     hin[:, k, ms * FD:(ms + 1) * FD],
                                     start=(k == 0), stop=(k == NT - 1))
                ot = opool.tile([2, FD], f32, name=f"o_{mc}_{ms}", tag="o")
                nc.scalar.activation(ot[:], ps4[:], Act.Sigmoid,
                                     bias=cst[0:2, 112:113])
                nc.sync.dma_start(
                    probsT_d.ap()[:, mc * MC + ms * FD:mc * MC + (ms + 1) * FD],
                    ot[:])

    nc.compile()
    return nc


def _prep_in_maps(inputs):
    x = np.asarray(inputs["x"], dtype=np.float32)
    sl = np.asarray(inputs["sample_loss"], dtype=np.float32)
    dm = np.asarray(inputs["demo_metric"], dtype=np.float32)
    alpha = np.asarray(inputs["alpha"], dtype=np.float32)
    b4 = np.asarray(inputs["b4"], dtype=np.float32)

    w1 = np.asarray(inputs["W1"], dtype=np.float32).astype(BF16)
    w2 = np.asarray(inputs["W2"], dtype=np.float32).astype(BF16)
    w3 = np.asarray(inputs["W3"], dtype=np.float32).astype(BF16)
    w4 = np.asarray(inputs["W4"], dtype=np.float32).astype(BF16)

    cst = np.zeros((P, 113), np.float32)
    for col, bname in ((0, "b1"), (32, "b2"), (64, "b3")):
        bv = np.asarray(inputs[bname], dtype=np.float32)
        cst[:, col:col + bv.size // P] = bv.reshape(-1, P).T
    cst[:, 96:112] = alpha.reshape(-1, P).T
    cst[0:2, 112] = b4

    in_maps = []
    for c in range(NCORES):
        xT = np.ascontiguousarray(x[c * M:(c + 1) * M].T).astype(BF16)
        slT = np.ascontiguousarray(sl[c * DS:(c + 1) * DS].T)
        dmT = np.ascontiguousarray(dm[c * DS:(c + 1) * DS].T)
        in_maps.append({
            "xT": xT, "w1": w1, "w2": w2, "w3": w3, "w4": w4,
            "cst": cst, "slT": slT, "dmT": dmT,
        })
    return in_maps


def kernel(**inputs):
    from concourse.bass_utils import run_bass_kernel_spmd

    if "nc" not in _CACHE:
        _CACHE["nc"] = _build()
    nc = _CACHE["nc"]

    in_maps = _prep_in_maps(inputs)
    res = run_bass_kernel_spmd(nc, in_maps, core_ids=list(range(NCORES)))

    probs = np.concatenate(
        [res.results[c]["probsT"].T for c in range(NCORES)], axis=0)
    part = sum(float(res.results[c]["lpart"].sum(dtype=np.float64))
               for c in range(NCORES))
    c0 = float(np.asarray(inputs["subdom_constant"]).reshape(-1)[0])
    loss = np.float32(part - c0 * D * K)
    return loss, probs.astype(np.float32)


# revision 15
# speedup vs baseline: 5499.7489x; 1.0058x over previous
"""Trainium2 Bass kernel for nn_Classifier (subdom hinge loss + 4-layer MLP).

Data-parallel over 8 NeuronCores: batch rows of x and demo rows of
sample_loss/demo_metric are sharded; MLP weights are replicated.

Device layout: activations kept transposed ([feature, batch]) so the weight
matrices are the natural stationary (lhsT) operand of the tensor engine and
no on-device transposes are needed.  Matmuls run in bf16 with fp32 PSUM
accumulation; the hinge loss runs in fp32 on the vector/scalar engines with
a fused relu+row-sum (accum_out).
"""

import numpy as np
import ml_dtypes
from contextlib import ExitStack

# Problem sizes (hardcoded per the task spec).
B, F, H = 16384, 2048, 4096
D, K = 4096, 2048
NCORES = 8
M = B // NCORES          # 2048 batch rows per core
DS = D // NCORES         # 512 demo rows per core
P = 128
FD = 512                 # matmul moving free dim == m-chunk size
NMC = M // FD            # 4 m-chunks
KT1 = F // P             # 16 k-tiles, layer 1
NT = H // P              # 32 n-tiles (and k-tiles for layers 2/3)
KLT = K // P             # 16 k-tiles of the demo feature dim

BF16 = ml_dtypes.bfloat16

_CACHE = {}


def _build(MC=1024, ms_inner=True, wbufs=3, xbufs=1, pbufs=4,
           split_x=True, loss_mid=True, use_fp16=True):
    import concourse.tile as tile
    from concourse import bacc, mybir

    NMCL = M // MC           # number of m-chunks
    MS = MC // FD            # 512-wide m-subchunks per chunk

    f32 = mybir.dt.float32
    # fp16 streams at the same 1 column/cycle as bf16 but carries 10 mantissa
    # bits instead of 7; all values here are O(10) so range is a non-issue.
    bf16 = mybir.dt.float16 if use_fp16 else mybir.dt.bfloat16
    Act = mybir.ActivationFunctionType

    nc = bacc.Bacc("TRN2", target_bir_lowering=False, debug=False,
                   num_devices=NCORES)

    xT_d = nc.dram_tensor("xT", [F, M], bf16, kind="ExternalInput")
    w1_d = nc.dram_tensor("w1", [F, H], bf16, kind="ExternalInput")
    w2_d = nc.dram_tensor("w2", [H, H], bf16, kind="ExternalInput")
    w3_d = nc.dram_tensor("w3", [H, H], bf16, kind="ExternalInput")
    w4_d = nc.dram_tensor("w4", [H, 2], bf16, kind="ExternalInput")
    # packed consts: cols 0:32 b1, 32:64 b2, 64:96 b3, 96:112 alpha, 112 b4
    cst_d = nc.dram_tensor("cst", [P, 113], f32, kind="ExternalInput")
    slT_d = nc.dram_tensor("slT", [K, DS], f32, kind="ExternalInput")
    dmT_d = nc.dram_tensor("dmT", [K, DS], f32, kind="ExternalInput")
    probsT_d = nc.dram_tensor("probsT", [2, M], f32, kind="ExternalOutput")
    lpart_d = nc.dram_tensor("lpart", [P, 1], f32, kind="ExternalOutput")

    with tile.TileContext(nc) as tc, ExitStack() as ctx:
        const = ctx.enter_context(tc.tile_pool(name="const", bufs=1))
        cst = const.tile([P, 113], f32, name="cst_sb")
        nc.sync.dma_start(cst[:], cst_d.ap()[:])
        w4t = const.tile([P, NT, 2], bf16, name="w4_sb")
        nc.sync.dma_start(w4t[:], w4_d.ap().rearrange("(kt p) n -> p kt n", p=P))
        acc = const.tile([P, KLT], f32, name="acc_sb")

        # ---- subdom hinge loss: sum over relu(alpha*(sl-dm)+1), fp32 ----
        lpool = ctx.enter_context(tc.tile_pool(name="loss", bufs=2))

        def emit_loss():
            for j in range(KLT):
                slt = lpool.tile([P, DS], f32, name=f"sl_{j}", tag="sl")
                nc.sync.dma_start(slt[:], slT_d.ap()[j * P:(j + 1) * P, :])
                dmt = lpool.tile([P, DS], f32, name=f"dm_{j}", tag="dm")
                nc.sync.dma_start(dmt[:], dmT_d.ap()[j * P:(j + 1) * P, :])
                nc.vector.tensor_sub(slt[:], slt[:], dmt[:])
                nc.scalar.activation(slt[:], slt[:], Act.Relu, bias=1.0,
                                     scale=cst[:, 96 + j:97 + j],
                                     accum_out=acc[:, j:j + 1])
            lsum = const.tile([P, 1], f32, name="lsum_sb")
            nc.vector.reduce_sum(lsum[:], acc[:], axis=mybir.AxisListType.X)
            nc.sync.dma_start(lpart_d.ap()[:], lsum[:])

        if not loss_mid:
            emit_loss()

        # ---- MLP: 4 m-chunks of 512 batch cols ----
        xpool = ctx.enter_context(tc.tile_pool(name="xp", bufs=xbufs))
        hpool = ctx.enter_context(tc.tile_pool(name="hp", bufs=2))
        wpool = ctx.enter_context(tc.tile_pool(name="wp", bufs=wbufs))
        ppool = ctx.enter_context(tc.tile_pool(name="pp", bufs=pbufs,
                                               space="PSUM"))
        p4pool = ctx.enter_context(tc.tile_pool(name="p4", bufs=2, space="PSUM"))
        opool = ctx.enter_context(tc.tile_pool(name="op", bufs=2))

        layers = [(w1_d, KT1, 0), (w2_d, NT, 32), (w3_d, NT, 64)]
        for mc in range(NMCL):
            if loss_mid and mc == 1:
                emit_loss()
            xt = xpool.tile([P, KT1, MC], bf16, name=f"x_{mc}", tag="x")
            xsrc = (xT_d.ap().rearrange("(kt p) m -> p kt m", p=P)
                    [:, :, mc * MC:(mc + 1) * MC])
            if split_x:
                for kt in range(KT1):
                    nc.sync.dma_start(xt[:, kt, :], xsrc[:, kt, :])
            else:
                nc.sync.dma_start(xt[:], xsrc)
            hin = xt
            for li, (w_d, KT, boff) in enumerate(layers):
                hout = hpool.tile([P, NT, MC], bf16, name=f"h_{mc}_{li}",
                                  tag="h")
                for n in range(NT):
                    wt = wpool.tile([P, KT, P], bf16, name=f"w_{mc}_{li}_{n}",
                                    tag="w")
                    nc.sync.dma_start(
                        wt[:],
                        w_d.ap().rearrange("(kt p) n -> p kt n", p=P)
                        [:, :, n * P:(n + 1) * P])
                    pss = [ppool.tile([P, FD], f32,
                                      name=f"ps_{mc}_{li}_{n}_{ms}", tag="ps")
                           for ms in range(MS)]
                    if ms_inner:
                        for k in range(KT):
                            for ms in range(MS):
                                nc.tensor.matmul(
                                    pss[ms][:], wt[:, k, :],
                                    hin[:, k, ms * FD:(ms + 1) * FD],
                                    start=(k == 0), stop=(k == KT - 1))
                    else:
                        for ms in range(MS):
                            for k in range(KT):
                                nc.tensor.matmul(
                                    pss[ms][:], wt[:, k, :],
                                    hin[:, k, ms * FD:(ms + 1) * FD],
                                    start=(k == 0), stop=(k == KT - 1))
                    for ms in range(MS):
                        nc.scalar.activation(
                            hout[:, n, ms * FD:(ms + 1) * FD], pss[ms][:],
                            Act.Relu, bias=cst[:, boff + n:boff + n + 1])
                hin = hout
            for ms in range(MS):
                ps4 = p4pool.tile([2, FD], f32, name=f"ps4_{mc}_{ms}",
                                  tag="ps4")
                for k in range(NT):
                    nc.tensor.matmul(ps4[:], w4t[:, k, :],
                                     hin[:, k, ms * FD:(ms + 1) * FD],
                                     start=(k == 0), stop=(k == NT - 1))
                ot = opool.tile([2, FD], f32, name=f"o_{mc}_{ms}", tag="o")
                nc.scalar.activation(ot[:], ps4[:], Act.Sigmoid,
                                     bias=cst[0:2, 112:113])
                nc.sync.dma_start(
                    probsT_d.ap()[:, mc * MC + ms * FD:mc * MC + (ms + 1) * FD],
                    ot[:])

    nc.compile()
    return nc


def _prep_in_maps(inputs, dt16=np.float16):
    global BF16
    BF16 = dt16
    x = np.asarray(inputs["x"], dtype=np.float32)
    sl = np.asarray(inputs["sample_loss"], dtype=np.float32)
    dm = np.asarray(inputs["demo_metric"], dtype=np.float32)
    alpha = np.asarray(inputs["alpha"], dtype=np.float32)
    b4 = np.asarray(inputs["b4"], dtype=np.float32)

    w1 = np.asarray(inputs["W1"], dtype=np.float32).astype(BF16)
    w2 = np.asarray(inputs["W2"], dtype=np.float32).astype(BF16)
    w3 = np.asarray(inputs["W3"], dtype=np.float32).astype(BF16)
    w4 = np.asarray(inputs["W4"], dtype=np.float32).astype(BF16)

    cst = np.zeros((P, 113), np.float32)
    for col, bname in ((0, "b1"), (32, "b2"), (64, "b3")):
        bv = np.asarray(inputs[bname], dtype=np.float32)
        cst[:, col:col + bv.size // P] = bv.reshape(-1, P).T
    cst[:, 96:112] = alpha.reshape(-1, P).T
    cst[0:2, 112] = b4

    in_maps = []
    for c in range(NCORES):
        xT = np.ascontiguousarray(x[c * M:(c + 1) * M].T).astype(BF16)
        slT = np.ascontiguousarray(sl[c * DS:(c + 1) * DS].T)
        dmT = np.ascontiguousarray(dm[c * DS:(c + 1) * DS].T)
        in_maps.append({
            "xT": xT, "w1": w1, "w2": w2, "w3": w3, "w4": w4,
            "cst": cst, "slT": slT, "dmT": dmT,
        })
    return in_maps


def kernel(**inputs):
    from concourse.bass_utils import run_bass_kernel_spmd

    if "nc" not in _CACHE:
        _CACHE["nc"] = _build()
    nc = _CACHE["nc"]

    in_maps = _prep_in_maps(inputs)
    res = run_bass_kernel_spmd(nc, in_maps, core_ids=list(range(NCORES)))

    probs = np.concatenate(
        [res.results[c]["probsT"].T for c in range(NCORES)], axis=0)
    part = sum(float(res.results[c]["lpart"].sum(dtype=np.float64))
               for c in range(NCORES))
    c0 = float(np.asarray(inputs["subdom_constant"]).reshape(-1)[0])
    loss = np.float32(part - c0 * D * K)
    return loss, probs.astype(np.float32)
